# revision 7
# baseline (speedup 1.0000x reference)
"""LoRA Linear kernel for 8x TRN2 NeuronCores (Bass/Tile).

Computes  y = x @ W^T + b + 2.0 * ((x @ A^T) @ B^T)   for
  x [4, 2048, 4096] f32, W [4096, 4096], b [4096], A [16, 4096], B [4096, 16].

Strategy (v3):
  - LoRA folded into the weight on the host: W_eff = W + 2*B@A (exact
    restructuring), so the device runs a single dense GEMM + bias.
  - Data-parallel over tokens: 8192 tokens -> 1024 per core.
  - Mixed precision contraction: KF8 of 32 k-tiles run as fp8e4m3 DoubleRow
    matmuls (2 k-tiles per instruction, ~2x rate); the rest run bf16.  The
    DoubleRow matmuls are interleaved between bf16 ones so their 256-column
    LDWEIGHTS hides under the previous matmul's streaming phase.
  - All weights are pre-scaled by 64 so fp8 W8 = fp8(64*W_eff) sits in e4m3
    normal range; PSUM accumulates 64*y in f32 and the ACT drain multiplies
    by 1/64.  The bias is added by the DVE during the drain (PE stays pure
    GEMM).
  - x is DMA'd in 8 token-chunks (prepacked contiguously on host); the first
    chunk and the first W block are issued first so the PE starts ~15us in.
"""

import os

import numpy as np
import ml_dtypes

_BF16 = ml_dtypes.bfloat16
_F8 = ml_dtypes.float8_e4m3

# Problem constants (hardcoded per harness contract).
_B, _S, _D, _O, _R = 4, 2048, 4096, 4096, 16
_T = _B * _S          # 8192 tokens
_NCORES = 8
_TC = _T // _NCORES   # 1024 tokens per core

P = 128
DS = _D // P          # 32 contraction k-tiles
KF8 = 8               # k-tiles done in fp8 DoubleRow (must be even)
KBF = DS - KF8        # k-tiles done in bf16
NTT = _TC // P        # 8 token-tiles per core
OBW = 512             # o-block width (one PSUM bank of f32)
NOB = _O // OBW       # 8 o-blocks
SCALE = 64.0          # global PSUM scale carried by the weights

_cache = {}

# Set by kernel() when KERNEL_TRACE=1; read by test.py for exec_time_ns.
LAST_RESULT = None


def _build_module():
    import concourse.bass as bass
    import concourse.bacc as bacc
    import concourse.mybir as mybir
    import concourse.tile as tile
    from concourse.bass import ts

    bf16 = mybir.dt.bfloat16
    f8 = mybir.dt.float8e4
    f32 = mybir.dt.float32
    DR = mybir.MatmulPerfMode.DoubleRow

    nc = bacc.Bacc("TRN2", target_bir_lowering=False, debug=False)
    # x prepacked into contiguous token-chunks: [tt][p][ds][128 tokens]
    xb_d = nc.dram_tensor("xb", [NTT, P, KBF, P], bf16, kind="ExternalInput")
    x8_d = nc.dram_tensor("x8", [NTT, P, KF8, P], f8, kind="ExternalInput")
    # W_eff prepacked into o-blocks: [ob][p][ds][512 outs]
    Wb_d = nc.dram_tensor("Wb", [NOB, P, KBF, OBW], bf16, kind="ExternalInput")
    W8_d = nc.dram_tensor("W8", [NOB, P, KF8, OBW], f8, kind="ExternalInput")
    # bias broadcast to all partitions: [P, O] f32 (unscaled)
    bias_d = nc.dram_tensor("bias", [P, _O], f32, kind="ExternalInput")
    out_d = nc.dram_tensor("out", [_TC, _O], f32, kind="ExternalOutput")

    with tile.TileContext(nc) as tc:
        with (
            tc.tile_pool(name="const", bufs=1) as cpool,
            tc.tile_pool(name="wpool_b", bufs=2) as wpool_b,
            tc.tile_pool(name="wpool_8", bufs=2) as wpool_8,
            tc.tile_pool(name="opool", bufs=6) as opool,
            tc.tile_pool(name="ps_mm", bufs=4, space="PSUM") as ps_pool,
            tc.tile_pool(name="ps_warm", bufs=1, space="PSUM") as ps_warm,
        ):
            xb_sb = cpool.tile([P, NTT, KBF, P], bf16)   # 48KB/partition
            x8_sb = cpool.tile([P, NTT, KF8, P], f8)     # 8KB/partition
            bias_sb = cpool.tile([P, _O], f32)           # 16KB/partition
            warm_sb = cpool.tile([P, P], bf16)

            # PE warm-up: ~60 matmuls on a zeroed tile keep the PE busy from
            # t~0 so the HAM clock-gate opens (K=8/8 after ~3.4us of
            # activity) before the first real operands arrive by DMA.  Their
            # PSUM bank is never read.
            nc.vector.memset(warm_sb[:], 0.0)
            ps_w = ps_warm.tile([P, P], f32)
            for _ in range(60):
                nc.tensor.matmul(
                    ps_w[:], warm_sb[:], warm_sb[:], start=True, stop=True
                )

            # DMA issue order tuned for a fast start: the operands of the
            # first DoubleRow matmuls first (all fp8 x chunks are tiny), then
            # the first bf16 x chunk, then the first W block split per k-tile
            # so the bf16 matmuls of the first group start progressively.
            for ob in range(NOB):
                Wb_blk = wpool_b.tile([P, KBF, OBW], bf16)
                W8_blk = wpool_8.tile([P, KF8, OBW], f8)
                nc.sync.dma_start(W8_blk[:], W8_d[ob, :, :, :])
                if ob == 0:
                    for tt in range(NTT):
                        nc.sync.dma_start(x8_sb[:, tt, :, :], x8_d[tt, :, :, :])
                    nc.sync.dma_start(xb_sb[:, 0, :, :], xb_d[0, :, :, :])
                    for ds in range(KBF):
                        nc.sync.dma_start(
                            Wb_blk[:, ds, :], Wb_d[ob, :, ds, :]
                        )
                    # Remaining input DMA, behind the first compute wave.
                    nc.sync.dma_start(bias_sb[:], bias_d[:, :])
                    for tt in range(1, NTT):
                        nc.sync.dma_start(xb_sb[:, tt, :, :], xb_d[tt, :, :, :])
                else:
                    nc.sync.dma_start(Wb_blk[:], Wb_d[ob, :, :, :])
                for tt in range(NTT):
                    ps = ps_pool.tile([P, OBW], f32)
                    # Interleave fp8 DoubleRow pairs between bf16 matmuls so
                    # the 256-col LDWEIGHTS of each DR hides under streaming.
                    # The very first group runs all DR pairs up front instead:
                    # they only need the small fp8 DMAs, buying time for the
                    # first W block to land.
                    seq = []
                    if ob == 0 and tt == 0:
                        for i in range(KF8 // 2):
                            seq.append(("dr", 2 * i))
                        for ds in range(KBF):
                            seq.append(("bf", ds))
                    else:
                        for i in range(KF8 // 2):
                            seq.append(("dr", 2 * i))
                            seq.append(("bf", i))
                        for ds in range(KF8 // 2, KBF):
                            seq.append(("bf", ds))
                    for j, (kind, idx) in enumerate(seq):
                        first = j == 0
                        last = j == len(seq) - 1
                        if kind == "dr":
                            nc.tensor.matmul(
                                ps[:],
                                x8_sb[:, tt, idx : idx + 2, :],
                                W8_blk[:, idx : idx + 2, :],
                                start=first,
                                stop=last,
                                perf_mode=DR,
                            )
                        else:
                            nc.tensor.matmul(
                                ps[:],
                                xb_sb[:, tt, idx, :],
                                Wb_blk[:, idx, :],
                                start=first,
                                stop=last,
                            )
                    if ob == NOB - 1 and tt == NTT - 1:
                        # Last group: drain in column slices so the final
                        # ACT -> DVE -> DMA chain pipelines instead of adding
                        # one long serial tail after the last matmul.
                        for j in range(4):
                            otj = opool.tile([P, OBW // 4], f32)
                            sl = slice(j * (OBW // 4), (j + 1) * (OBW // 4))
                            nc.scalar.mul(otj[:], ps[:, sl], 1.0 / SCALE)
                            nc.vector.tensor_add(
                                otj[:],
                                otj[:],
                                bias_sb[:, ob * OBW + j * (OBW // 4) :][
                                    :, : OBW // 4
                                ],
                            )
                            nc.sync.dma_start(
                                out_d[ts(tt, P), ob * OBW + j * (OBW // 4) :][
                                    :, : OBW // 4
                                ],
                                otj[:],
                            )
                    else:
                        ot = opool.tile([P, OBW], f32)
                        nc.scalar.mul(ot[:], ps[:], 1.0 / SCALE)
                        nc.vector.tensor_add(
                            ot[:], ot[:], bias_sb[:, ts(ob, OBW)]
                        )
                        nc.sync.dma_start(
                            out_d[ts(tt, P), ts(ob, OBW)], ot[:]
                        )
    nc.compile()
    return nc


def _prep_inputs(x, W, b, lora_A, lora_B):
    """Host-side weight prep: fold LoRA, transpose, scale, split precision."""
    Weff = (W + 2.0 * (lora_B @ lora_A)).astype(np.float32)  # [O, D]
    WT = np.ascontiguousarray(Weff.T) * SCALE                # [D, O], x64

    # W blocks: [NOB][P][DS][OBW]; k-tile ds occupies rows ds*128:(ds+1)*128.
    W4 = WT.reshape(DS, P, NOB, OBW)                         # [ds][p][ob][obw]
    W8 = np.ascontiguousarray(
        W4[:KF8].transpose(2, 1, 0, 3)                       # [ob][p][ds8][obw]
    ).astype(_F8)
    Wb = np.ascontiguousarray(
        W4[KF8:].transpose(2, 1, 0, 3)                       # [ob][p][ds24][obw]
    ).astype(_BF16)

    xf = np.ascontiguousarray(x.reshape(_T, _D))             # [T, D]
    bias = np.broadcast_to(b.astype(np.float32), (P, _O)).copy()
    return xf, Wb, W8, bias


def kernel(x, W, b, lora_A, lora_B):
    global LAST_RESULT
    from concourse.bass_utils import run_bass_kernel_spmd

    if "nc" not in _cache:
        _cache["nc"] = _build_module()
    nc = _cache["nc"]

    xf, Wb, W8, bias = _prep_inputs(x, W, b, lora_A, lora_B)

    in_maps = []
    for c in range(_NCORES):
        xc = xf[c * _TC : (c + 1) * _TC]                     # [TC, D]
        # xT chunks: [tt][p][ds][128 tokens] with k-tile ds = rows ds*128...
        xT = xc.T.reshape(DS, P, NTT, P)                     # [ds][p][tt][t]
        x8c = np.ascontiguousarray(
            xT[:KF8].transpose(2, 1, 0, 3)                   # [tt][p][ds8][t]
        ).astype(_F8)
        xbc = np.ascontiguousarray(
            xT[KF8:].transpose(2, 1, 0, 3)                   # [tt][p][ds24][t]
        ).astype(_BF16)
        in_maps.append(
            {
                "xb": xbc,
                "x8": x8c,
                "Wb": Wb,
                "W8": W8,
                "bias": bias,
            }
        )

    trace = os.environ.get("KERNEL_TRACE", "0") == "1"
    res = run_bass_kernel_spmd(
        nc,
        in_maps,
        core_ids=list(range(_NCORES)),
        trace=trace,
    )
    LAST_RESULT = res

    out = np.concatenate([r["out"] for r in res.results], axis=0)
    return out.reshape(_B, _S, _O).astype(np.float32, copy=False)


# revision 10
# speedup vs baseline: 1.0130x; 1.0130x over previous
"""LoRA Linear kernel for 8x TRN2 NeuronCores (Bass/Tile).

Computes  y = x @ W^T + b + 2.0 * ((x @ A^T) @ B^T)   for
  x [4, 2048, 4096] f32, W [4096, 4096], b [4096], A [16, 4096], B [4096, 16].

Strategy (v3):
  - LoRA folded into the weight on the host: W_eff = W + 2*B@A (exact
    restructuring), so the device runs a single dense GEMM + bias.
  - Data-parallel over tokens: 8192 tokens -> 1024 per core.
  - Mixed precision contraction: KF8 of 32 k-tiles run as fp8e4m3 DoubleRow
    matmuls (2 k-tiles per instruction, ~2x rate); the rest run bf16.  The
    DoubleRow matmuls are interleaved between bf16 ones so their 256-column
    LDWEIGHTS hides under the previous matmul's streaming phase.
  - All weights are pre-scaled by 64 so fp8 W8 = fp8(64*W_eff) sits in e4m3
    normal range; PSUM accumulates 64*y in f32 and the ACT drain multiplies
    by 1/64.  The bias is added by the DVE during the drain (PE stays pure
    GEMM).
  - x is DMA'd in 8 token-chunks (prepacked contiguously on host); the first
    chunk and the first W block are issued first so the PE starts ~15us in.
"""

import os

import numpy as np
import ml_dtypes

_BF16 = ml_dtypes.bfloat16
_F8 = ml_dtypes.float8_e4m3

# Problem constants (hardcoded per harness contract).
_B, _S, _D, _O, _R = 4, 2048, 4096, 4096, 16
_T = _B * _S          # 8192 tokens
_NCORES = 8
_TC = _T // _NCORES   # 1024 tokens per core

P = 128
DS = _D // P          # 32 contraction k-tiles
KF8 = 8               # k-tiles done in fp8 DoubleRow (must be even)
KBF = DS - KF8        # k-tiles done in bf16
NTT = _TC // P        # 8 token-tiles per core
OBW = 512             # o-block width (one PSUM bank of f32)
NOB = _O // OBW       # 8 o-blocks
SCALE = 64.0          # global PSUM scale carried by the weights

_cache = {}

# Set by kernel() when KERNEL_TRACE=1; read by test.py for exec_time_ns.
LAST_RESULT = None


def _build_module():
    import concourse.bass as bass
    import concourse.bacc as bacc
    import concourse.mybir as mybir
    import concourse.tile as tile
    from concourse.bass import ts

    bf16 = mybir.dt.bfloat16
    f8 = mybir.dt.float8e4
    f32 = mybir.dt.float32
    DR = mybir.MatmulPerfMode.DoubleRow

    nc = bacc.Bacc("TRN2", target_bir_lowering=False, debug=False)
    # x prepacked into contiguous token-chunks: [tt][p][ds][128 tokens]
    xb_d = nc.dram_tensor("xb", [NTT, P, KBF, P], bf16, kind="ExternalInput")
    x8_d = nc.dram_tensor("x8", [NTT, P, KF8, P], f8, kind="ExternalInput")
    # W_eff prepacked into o-blocks: [ob][p][ds][512 outs]
    Wb_d = nc.dram_tensor("Wb", [NOB, P, KBF, OBW], bf16, kind="ExternalInput")
    W8_d = nc.dram_tensor("W8", [NOB, P, KF8, OBW], f8, kind="ExternalInput")
    # bias broadcast to all partitions: [P, O] f32 (unscaled)
    bias_d = nc.dram_tensor("bias", [P, _O], f32, kind="ExternalInput")
    warm_d = nc.dram_tensor("warm", [P, P], bf16, kind="ExternalInput")
    out_d = nc.dram_tensor("out", [_TC, _O], f32, kind="ExternalOutput")

    with tile.TileContext(nc) as tc:
        with (
            tc.tile_pool(name="const", bufs=1) as cpool,
            tc.tile_pool(name="wpool_b", bufs=2) as wpool_b,
            tc.tile_pool(name="wpool_8", bufs=2) as wpool_8,
            tc.tile_pool(name="opool", bufs=6) as opool,
            tc.tile_pool(name="ps_mm", bufs=4, space="PSUM") as ps_pool,
            tc.tile_pool(name="ps_warm", bufs=1, space="PSUM") as ps_warm,
        ):
            xb_sb = cpool.tile([P, NTT, KBF, P], bf16)   # 48KB/partition
            x8_sb = cpool.tile([P, NTT, KF8, P], f8)     # 8KB/partition
            bias_sb = cpool.tile([P, _O], f32)           # 16KB/partition
            warm_sb = cpool.tile([P, P], bf16)

            # PE warm-up: a tiny 32KB DMA (first in program order, lands as
            # soon as the DMA engines spin up ~6us) feeds ~80 matmuls that
            # keep the PE busy until the real operands arrive ~13us, so the
            # HAM clock-gate is open (K=8/8) when real work starts.  Their
            # PSUM bank is never read.
            nc.sync.dma_start(warm_sb[:], warm_d[:, :])
            ps_w = ps_warm.tile([P, P], f32)
            for _ in range(80):
                nc.tensor.matmul(
                    ps_w[:], warm_sb[:], warm_sb[:], start=True, stop=True
                )

            # DMA issue order tuned for a fast start: the operands of the
            # first group first, with the first W block split per k-tile so
            # the bf16 matmuls of the first group start progressively.
            nc.sync.dma_start(x8_sb[:, 0, :, :], x8_d[0, :, :, :])
            nc.sync.dma_start(xb_sb[:, 0, :, :], xb_d[0, :, :, :])

            for ob in range(NOB):
                Wb_blk = wpool_b.tile([P, KBF, OBW], bf16)
                W8_blk = wpool_8.tile([P, KF8, OBW], f8)
                nc.sync.dma_start(W8_blk[:], W8_d[ob, :, :, :])
                if ob == 0:
                    for ds in range(KBF):
                        nc.sync.dma_start(
                            Wb_blk[:, ds, :], Wb_d[ob, :, ds, :]
                        )
                    # Remaining input DMA, behind the first compute wave.
                    for tt in range(1, NTT):
                        nc.sync.dma_start(x8_sb[:, tt, :, :], x8_d[tt, :, :, :])
                        nc.sync.dma_start(xb_sb[:, tt, :, :], xb_d[tt, :, :, :])
                    nc.sync.dma_start(bias_sb[:], bias_d[:, :])
                else:
                    nc.sync.dma_start(Wb_blk[:], Wb_d[ob, :, :, :])
                for tt in range(NTT):
                    ps = ps_pool.tile([P, OBW], f32)
                    # Interleave fp8 DoubleRow pairs between bf16 matmuls so
                    # the 256-col LDWEIGHTS of each DR hides under streaming.
                    # The very first group runs all DR pairs up front instead:
                    # they only need the small fp8 DMAs, buying time for the
                    # first W block to land.
                    seq = []
                    if ob == 0 and tt == 0:
                        for i in range(KF8 // 2):
                            seq.append(("dr", 2 * i))
                        for ds in range(KBF):
                            seq.append(("bf", ds))
                    else:
                        for i in range(KF8 // 2):
                            seq.append(("dr", 2 * i))
                            seq.append(("bf", i))
                        for ds in range(KF8 // 2, KBF):
                            seq.append(("bf", ds))
                    for j, (kind, idx) in enumerate(seq):
                        first = j == 0
                        last = j == len(seq) - 1
                        if kind == "dr":
                            nc.tensor.matmul(
                                ps[:],
                                x8_sb[:, tt, idx : idx + 2, :],
                                W8_blk[:, idx : idx + 2, :],
                                start=first,
                                stop=last,
                                perf_mode=DR,
                            )
                        else:
                            nc.tensor.matmul(
                                ps[:],
                                xb_sb[:, tt, idx, :],
                                Wb_blk[:, idx, :],
                                start=first,
                                stop=last,
                            )
                    if ob == NOB - 1 and tt == NTT - 1:
                        # Last group: drain in column slices so the final
                        # ACT -> DVE -> DMA chain pipelines instead of adding
                        # one long serial tail after the last matmul.
                        for j in range(4):
                            otj = opool.tile([P, OBW // 4], f32)
                            sl = slice(j * (OBW // 4), (j + 1) * (OBW // 4))
                            nc.scalar.mul(otj[:], ps[:, sl], 1.0 / SCALE)
                            nc.vector.tensor_add(
                                otj[:],
                                otj[:],
                                bias_sb[:, ob * OBW + j * (OBW // 4) :][
                                    :, : OBW // 4
                                ],
                            )
                            nc.sync.dma_start(
                                out_d[ts(tt, P), ob * OBW + j * (OBW // 4) :][
                                    :, : OBW // 4
                                ],
                                otj[:],
                            )
                    else:
                        ot = opool.tile([P, OBW], f32)
                        nc.scalar.mul(ot[:], ps[:], 1.0 / SCALE)
                        nc.vector.tensor_add(
                            ot[:], ot[:], bias_sb[:, ts(ob, OBW)]
                        )
                        nc.sync.dma_start(
                            out_d[ts(tt, P), ts(ob, OBW)], ot[:]
                        )
    nc.compile()
    return nc


def _prep_inputs(x, W, b, lora_A, lora_B):
    """Host-side weight prep: fold LoRA, transpose, scale, split precision."""
    Weff = (W + 2.0 * (lora_B @ lora_A)).astype(np.float32)  # [O, D]
    WT = np.ascontiguousarray(Weff.T) * SCALE                # [D, O], x64

    # W blocks: [NOB][P][DS][OBW]; k-tile ds occupies rows ds*128:(ds+1)*128.
    W4 = WT.reshape(DS, P, NOB, OBW)                         # [ds][p][ob][obw]
    W8 = np.ascontiguousarray(
        W4[:KF8].transpose(2, 1, 0, 3)                       # [ob][p][ds8][obw]
    ).astype(_F8)
    Wb = np.ascontiguousarray(
        W4[KF8:].transpose(2, 1, 0, 3)                       # [ob][p][ds24][obw]
    ).astype(_BF16)

    xf = np.ascontiguousarray(x.reshape(_T, _D))             # [T, D]
    bias = np.broadcast_to(b.astype(np.float32), (P, _O)).copy()
    return xf, Wb, W8, bias


def kernel(x, W, b, lora_A, lora_B):
    global LAST_RESULT
    from concourse.bass_utils import run_bass_kernel_spmd

    if "nc" not in _cache:
        _cache["nc"] = _build_module()
    nc = _cache["nc"]

    xf, Wb, W8, bias = _prep_inputs(x, W, b, lora_A, lora_B)

    in_maps = []
    for c in range(_NCORES):
        xc = xf[c * _TC : (c + 1) * _TC]                     # [TC, D]
        # xT chunks: [tt][p][ds][128 tokens] with k-tile ds = rows ds*128...
        xT = xc.T.reshape(DS, P, NTT, P)                     # [ds][p][tt][t]
        x8c = np.ascontiguousarray(
            xT[:KF8].transpose(2, 1, 0, 3)                   # [tt][p][ds8][t]
        ).astype(_F8)
        xbc = np.ascontiguousarray(
            xT[KF8:].transpose(2, 1, 0, 3)                   # [tt][p][ds24][t]
        ).astype(_BF16)
        in_maps.append(
            {
                "xb": xbc,
                "x8": x8c,
                "Wb": Wb,
                "W8": W8,
                "bias": bias,
                "warm": np.zeros((P, P), dtype=_BF16),
            }
        )

    trace = os.environ.get("KERNEL_TRACE", "0") == "1"
    res = run_bass_kernel_spmd(
        nc,
        in_maps,
        core_ids=list(range(_NCORES)),
        trace=trace,
    )
    LAST_RESULT = res

    out = np.concatenate([r["out"] for r in res.results], axis=0)
    return out.reshape(_B, _S, _O).astype(np.float32, copy=False)


# revision 11
# speedup vs baseline: 1.0243x; 1.0112x over previous
"""LoRA Linear kernel for 8x TRN2 NeuronCores (Bass/Tile).

Computes  y = x @ W^T + b + 2.0 * ((x @ A^T) @ B^T)   for
  x [4, 2048, 4096] f32, W [4096, 4096], b [4096], A [16, 4096], B [4096, 16].

Strategy (v3):
  - LoRA folded into the weight on the host: W_eff = W + 2*B@A (exact
    restructuring), so the device runs a single dense GEMM + bias.
  - Data-parallel over tokens: 8192 tokens -> 1024 per core.
  - Mixed precision contraction: KF8 of 32 k-tiles run as fp8e4m3 DoubleRow
    matmuls (2 k-tiles per instruction, ~2x rate); the rest run bf16.  The
    DoubleRow matmuls are interleaved between bf16 ones so their 256-column
    LDWEIGHTS hides under the previous matmul's streaming phase.
  - All weights are pre-scaled by 64 so fp8 W8 = fp8(64*W_eff) sits in e4m3
    normal range; PSUM accumulates 64*y in f32 and the ACT drain multiplies
    by 1/64.  The bias is added by the DVE during the drain (PE stays pure
    GEMM).
  - x is DMA'd in 8 token-chunks (prepacked contiguously on host); the first
    chunk and the first W block are issued first so the PE starts ~15us in.
"""

import os

import numpy as np
import ml_dtypes

_BF16 = ml_dtypes.bfloat16
_F8 = ml_dtypes.float8_e4m3

# Problem constants (hardcoded per harness contract).
_B, _S, _D, _O, _R = 4, 2048, 4096, 4096, 16
_T = _B * _S          # 8192 tokens
_NCORES = 8
_TC = _T // _NCORES   # 1024 tokens per core

P = 128
DS = _D // P          # 32 contraction k-tiles
KF8 = 8               # k-tiles done in fp8 DoubleRow (must be even)
KBF = DS - KF8        # k-tiles done in bf16
NTT = _TC // P        # 8 token-tiles per core
OBW = 512             # o-block width (one PSUM bank of f32)
NOB = _O // OBW       # 8 o-blocks
SCALE = 64.0          # global PSUM scale carried by the weights

_cache = {}

# Set by kernel() when KERNEL_TRACE=1; read by test.py for exec_time_ns.
LAST_RESULT = None


def _build_module():
    import concourse.bass as bass
    import concourse.bacc as bacc
    import concourse.mybir as mybir
    import concourse.tile as tile
    from concourse.bass import ts

    bf16 = mybir.dt.bfloat16
    f8 = mybir.dt.float8e4
    f32 = mybir.dt.float32
    DR = mybir.MatmulPerfMode.DoubleRow

    nc = bacc.Bacc("TRN2", target_bir_lowering=False, debug=False)
    # x prepacked into contiguous token-chunks: [tt][p][ds][128 tokens]
    xb_d = nc.dram_tensor("xb", [NTT, P, KBF, P], bf16, kind="ExternalInput")
    x8_d = nc.dram_tensor("x8", [NTT, P, KF8, P], f8, kind="ExternalInput")
    # W_eff prepacked into o-blocks: [ob][p][ds][512 outs]
    Wb_d = nc.dram_tensor("Wb", [NOB, P, KBF, OBW], bf16, kind="ExternalInput")
    W8_d = nc.dram_tensor("W8", [NOB, P, KF8, OBW], f8, kind="ExternalInput")
    # bias broadcast to all partitions: [P, O] f32 (unscaled)
    bias_d = nc.dram_tensor("bias", [P, _O], bf16, kind="ExternalInput")
    warm_d = nc.dram_tensor("warm", [P, P], bf16, kind="ExternalInput")
    out_d = nc.dram_tensor("out", [_TC, _O], f32, kind="ExternalOutput")

    with tile.TileContext(nc) as tc:
        with (
            tc.tile_pool(name="const", bufs=1) as cpool,
            tc.tile_pool(name="wpool_b", bufs=2) as wpool_b,
            tc.tile_pool(name="wpool_8", bufs=2) as wpool_8,
            tc.tile_pool(name="opool", bufs=6) as opool,
            tc.tile_pool(name="ps_mm", bufs=4, space="PSUM") as ps_pool,
            tc.tile_pool(name="ps_warm", bufs=1, space="PSUM") as ps_warm,
        ):
            xb_sb = cpool.tile([P, NTT, KBF, P], bf16)   # 48KB/partition
            x8_sb = cpool.tile([P, NTT, KF8, P], f8)     # 8KB/partition
            bias_sb = cpool.tile([P, _O], bf16)          # 8KB/partition
            warm_sb = cpool.tile([P, P], bf16)

            # PE warm-up: a tiny 32KB DMA (first in program order, lands as
            # soon as the DMA engines spin up ~6us) feeds ~80 matmuls that
            # keep the PE busy until the real operands arrive ~13us, so the
            # HAM clock-gate is open (K=8/8) when real work starts.  Their
            # PSUM bank is never read.
            nc.sync.dma_start(warm_sb[:], warm_d[:, :])
            ps_w = ps_warm.tile([P, P], f32)
            for _ in range(48):
                nc.tensor.matmul(
                    ps_w[:], warm_sb[:], warm_sb[:], start=True, stop=True
                )

            # DMA issue order tuned for a fast start: the operands of the
            # first group first, with the first W block split per k-tile so
            # the bf16 matmuls of the first group start progressively.
            nc.sync.dma_start(x8_sb[:, 0, :, :], x8_d[0, :, :, :])
            nc.sync.dma_start(xb_sb[:, 0, :, :], xb_d[0, :, :, :])

            for ob in range(NOB):
                Wb_blk = wpool_b.tile([P, KBF, OBW], bf16)
                W8_blk = wpool_8.tile([P, KF8, OBW], f8)
                nc.scalar.dma_start(W8_blk[:], W8_d[ob, :, :, :])
                if ob == 0:
                    for ds in range(KBF):
                        nc.scalar.dma_start(
                            Wb_blk[:, ds, :], Wb_d[ob, :, ds, :]
                        )
                    # Remaining input DMA, behind the first compute wave.
                    for tt in range(1, NTT):
                        nc.sync.dma_start(x8_sb[:, tt, :, :], x8_d[tt, :, :, :])
                        nc.sync.dma_start(xb_sb[:, tt, :, :], xb_d[tt, :, :, :])
                    nc.sync.dma_start(bias_sb[:], bias_d[:, :])
                else:
                    nc.scalar.dma_start(Wb_blk[:], Wb_d[ob, :, :, :])
                for tt in range(NTT):
                    ps = ps_pool.tile([P, OBW], f32)
                    # Interleave fp8 DoubleRow pairs between bf16 matmuls so
                    # the 256-col LDWEIGHTS of each DR hides under streaming.
                    # The very first group runs all DR pairs up front instead:
                    # they only need the small fp8 DMAs, buying time for the
                    # first W block to land.
                    seq = []
                    if ob == 0 and tt == 0:
                        for i in range(KF8 // 2):
                            seq.append(("dr", 2 * i))
                        for ds in range(KBF):
                            seq.append(("bf", ds))
                    else:
                        for i in range(KF8 // 2):
                            seq.append(("dr", 2 * i))
                            seq.append(("bf", i))
                        for ds in range(KF8 // 2, KBF):
                            seq.append(("bf", ds))
                    for j, (kind, idx) in enumerate(seq):
                        first = j == 0
                        last = j == len(seq) - 1
                        if kind == "dr":
                            nc.tensor.matmul(
                                ps[:],
                                x8_sb[:, tt, idx : idx + 2, :],
                                W8_blk[:, idx : idx + 2, :],
                                start=first,
                                stop=last,
                                perf_mode=DR,
                            )
                        else:
                            nc.tensor.matmul(
                                ps[:],
                                xb_sb[:, tt, idx, :],
                                Wb_blk[:, idx, :],
                                start=first,
                                stop=last,
                            )
                    if ob == NOB - 1 and tt == NTT - 1:
                        # Last group: drain in column slices so the final
                        # ACT -> DVE -> DMA chain pipelines instead of adding
                        # one long serial tail after the last matmul.
                        for j in range(4):
                            otj = opool.tile([P, OBW // 4], f32)
                            sl = slice(j * (OBW // 4), (j + 1) * (OBW // 4))
                            nc.scalar.mul(otj[:], ps[:, sl], 1.0 / SCALE)
                            nc.vector.tensor_add(
                                otj[:],
                                otj[:],
                                bias_sb[:, ob * OBW + j * (OBW // 4) :][
                                    :, : OBW // 4
                                ],
                            )
                            nc.sync.dma_start(
                                out_d[ts(tt, P), ob * OBW + j * (OBW // 4) :][
                                    :, : OBW // 4
                                ],
                                otj[:],
                            )
                    else:
                        ot = opool.tile([P, OBW], f32)
                        nc.scalar.mul(ot[:], ps[:], 1.0 / SCALE)
                        nc.vector.tensor_add(
                            ot[:], ot[:], bias_sb[:, ts(ob, OBW)]
                        )
                        nc.sync.dma_start(
                            out_d[ts(tt, P), ts(ob, OBW)], ot[:]
                        )
    nc.compile()
    return nc


def _prep_inputs(x, W, b, lora_A, lora_B):
    """Host-side weight prep: fold LoRA, transpose, scale, split precision."""
    Weff = (W + 2.0 * (lora_B @ lora_A)).astype(np.float32)  # [O, D]
    WT = np.ascontiguousarray(Weff.T) * SCALE                # [D, O], x64

    # W blocks: [NOB][P][DS][OBW]; k-tile ds occupies rows ds*128:(ds+1)*128.
    W4 = WT.reshape(DS, P, NOB, OBW)                         # [ds][p][ob][obw]
    W8 = np.ascontiguousarray(
        W4[:KF8].transpose(2, 1, 0, 3)                       # [ob][p][ds8][obw]
    ).astype(_F8)
    Wb = np.ascontiguousarray(
        W4[KF8:].transpose(2, 1, 0, 3)                       # [ob][p][ds24][obw]
    ).astype(_BF16)

    xf = np.ascontiguousarray(x.reshape(_T, _D))             # [T, D]
    bias = np.broadcast_to(b.astype(_BF16), (P, _O)).copy()
    return xf, Wb, W8, bias


def kernel(x, W, b, lora_A, lora_B):
    global LAST_RESULT
    from concourse.bass_utils import run_bass_kernel_spmd

    if "nc" not in _cache:
        _cache["nc"] = _build_module()
    nc = _cache["nc"]

    xf, Wb, W8, bias = _prep_inputs(x, W, b, lora_A, lora_B)

    in_maps = []
    for c in range(_NCORES):
        xc = xf[c * _TC : (c + 1) * _TC]                     # [TC, D]
        # xT chunks: [tt][p][ds][128 tokens] with k-tile ds = rows ds*128...
        xT = xc.T.reshape(DS, P, NTT, P)                     # [ds][p][tt][t]
        x8c = np.ascontiguousarray(
            xT[:KF8].transpose(2, 1, 0, 3)                   # [tt][p][ds8][t]
        ).astype(_F8)
        xbc = np.ascontiguousarray(
            xT[KF8:].transpose(2, 1, 0, 3)                   # [tt][p][ds24][t]
        ).astype(_BF16)
        in_maps.append(
            {
                "xb": xbc,
                "x8": x8c,
                "Wb": Wb,
                "W8": W8,
                "bias": bias,
                "warm": np.zeros((P, P), dtype=_BF16),
            }
        )

    trace = os.environ.get("KERNEL_TRACE", "0") == "1"
    res = run_bass_kernel_spmd(
        nc,
        in_maps,
        core_ids=list(range(_NCORES)),
        trace=trace,
    )
    LAST_RESULT = res

    out = np.concatenate([r["out"] for r in res.results], axis=0)
    return out.reshape(_B, _S, _O).astype(np.float32, copy=False)
